# revision 35
# baseline (speedup 1.0000x reference)
"""Local (sliding-window) self-attention Bass kernel for 8 TRN2 NeuronCores.

Problem: B=4, T=4096, C=512, H=8 heads, head_dim=64, window=15.
Sharding: 8 cores = batch(4) x seq-halves(2). Each core processes 2048 query
tokens of one batch element; its x chunk carries a 7-token halo on each side
(zero-padded at sequence edges, matching the reference's jnp.pad semantics),
padded to 2112 rows so the per-block k-tail chunk (64 wide) stays in range.

Host marshalling: x arrives pre-transposed (feature-major, bf16) and the
weights pre-cast to bf16 and pre-split, so the device only DMAs, applies the
mask, and runs matmuls.

Device dataflow (per core; all matmuls bf16 with fp32 PSUM):
  xT [128, 4*NKV] <- DMA; xT *= maskB (4 DVE ops, mask broadcast to 128 rows)
  qT/kT = W-stationary matmuls + bias (feature-major, via ACT copy)
  v_tok = xT-stationary matmuls + bias, token-major, packed [64 v | 1 ones]
          per head (the ones column makes AV emit the softmax denominator)
  per 128-query block i (queries qb=128i..qb+128):
    scoresT [k, q] per head: A chunk k in [qb, qb+128) (one matmul per head,
      even heads -> PSUM bank 0, odd -> bank 1: a matmul's tile_position row
      must be uniform within a bank), B tail k in [qb+128, qb+192) for
      queries qb+64.. (bank 2 even / bank 3 odd heads)
    additive band mask accumulated on the PE (band @ [I I I I]) drives
      out-of-band scores to -2000 so exp underflows to exactly 0
    exp per bank -> alpha bf16 (no DVE masking needed)
    AV token-major: av[q, 65] per head = alpha.T @ v_aug (den in col 64)
    reciprocal(den) * query-mask -> one DVE normalize -> avn bf16
    4 PE transposes -> attnT -> one ACT copy -> aT
    proj: aT-stationary matmuls; out = (bproj*mask) + psum via one DVE op
"""

import math
import os
from contextlib import ExitStack

import ml_dtypes
import numpy as np

import concourse.bacc as bacc
import concourse.bass as bass
import concourse.mybir as mybir
import concourse.tile as tile
from concourse import bass_utils

B, T, C, H, WIN = 4, 4096, 512, 8, 15
D = C // H            # 64
PAD = WIN // 2        # 7
NTOK = T // 2         # 2048 query tokens per core
NKV = 2112            # kv rows per core: 7 + 2048 + 7 = 2062, padded to 2112
NB = NTOK // 128      # 16 query blocks
SCALE = math.log(WIN) / D
F32 = mybir.dt.float32
BF16 = mybir.dt.bfloat16
M0 = 2000.0   # additive score mask; SCALE*M0 ~ 85 so exp underflows to 0


def _mask_consts() -> dict:
    """Additive band masks (transposed, as matmul lhsT) and replicated
    identities used to broadcast them across the head-columns of a bank.

    A-chunk in-band: k-14 <= q <= k.  B-tail (k=128+k'): q64 >= 50+k'
    where q = qb+64+q64.  B operands are zero-padded to K=128 so the mask
    matmul shares the full-array row config of the A-mask.
    """
    k = np.arange(128)[:, None]
    q = np.arange(128)[None, :]
    a = np.where((q >= k - 14) & (q <= k), 0.0, -M0).astype(np.float32)
    kp = np.arange(64)[:, None]
    q64 = np.arange(64)[None, :]
    b = np.where(q64 >= 50 + kp, 0.0, -M0).astype(np.float32)
    identj = np.zeros((128, 512), np.float32)
    for j in range(4):
        identj[:, j * 128:(j + 1) * 128] = np.eye(128)
    identj32 = np.zeros((128, 256), np.float32)
    for j in range(4):
        identj32[0:64, j * 64:(j + 1) * 64] = np.eye(64)
    return {
        "bandat": np.ascontiguousarray(a.T.astype(ml_dtypes.bfloat16)),
        "bandbt": np.ascontiguousarray(
            np.vstack([b.T, np.zeros((64, 64), np.float32)])
              .astype(ml_dtypes.bfloat16)),
        "identj": np.ascontiguousarray(identj.astype(ml_dtypes.bfloat16)),
        "identj32": np.ascontiguousarray(identj32.astype(ml_dtypes.bfloat16)),
    }


def _identity() -> np.ndarray:
    return np.eye(128, dtype=ml_dtypes.bfloat16)


def build_program() -> bacc.Bacc:
    nc = bacc.Bacc("TRN2", target_bir_lowering=False, debug=False,
                   enable_asserts=False, num_devices=8)

    xtd = nc.dram_tensor("xt", [C, NKV], BF16, kind="ExternalInput").ap()
    maskd = nc.dram_tensor("mask", [NKV], F32, kind="ExternalInput").ap()
    maskbd = nc.dram_tensor("maskb", [NKV], BF16, kind="ExternalInput").ap()
    wqd = nc.dram_tensor("wq", [C, C], BF16, kind="ExternalInput").ap()
    bqd = nc.dram_tensor("bq", [C], F32, kind="ExternalInput").ap()
    wkd = nc.dram_tensor("wk", [C, C], BF16, kind="ExternalInput").ap()
    wvd = nc.dram_tensor("wv", [C, C], BF16, kind="ExternalInput").ap()
    bkvd = nc.dram_tensor("bkv", [2 * C], F32, kind="ExternalInput").ap()
    wpd = nc.dram_tensor("wproj", [C, C], BF16, kind="ExternalInput").ap()
    bpd = nc.dram_tensor("bproj", [C], F32, kind="ExternalInput").ap()
    bandatd = nc.dram_tensor("bandat", [128, 128], BF16, kind="ExternalInput").ap()
    bandbtd = nc.dram_tensor("bandbt", [128, 64], BF16, kind="ExternalInput").ap()
    identjd = nc.dram_tensor("identj", [128, 512], BF16, kind="ExternalInput").ap()
    identj32d = nc.dram_tensor("identj32", [128, 256], BF16, kind="ExternalInput").ap()
    identd = nc.dram_tensor("ident", [128, 128], BF16, kind="ExternalInput").ap()
    outd = nc.dram_tensor("out", [NTOK, C], F32, kind="ExternalOutput").ap()

    with tile.TileContext(nc) as tc, ExitStack() as ctx:
        sb = ctx.enter_context(tc.tile_pool(name="sb", bufs=1))
        sb_a = ctx.enter_context(tc.tile_pool(name="sb_a", bufs=3))
        sb_o = ctx.enter_context(tc.tile_pool(name="sb_o", bufs=3))
        pp_sc = ctx.enter_context(tc.tile_pool(name="pp_sc", bufs=1, space="PSUM"))
        pp_tr = ctx.enter_context(tc.tile_pool(name="pp_tr", bufs=1, space="PSUM"))
        pp_pr = ctx.enter_context(tc.tile_pool(name="pp_pr", bufs=1, space="PSUM"))
        pp_av = ctx.enter_context(tc.tile_pool(name="pp_av", bufs=1, space="PSUM"))

        # ---- persistent SBUF tensors ----
        xT = sb.tile([128, 4 * NKV], BF16, tag="xT")     # col ci*NKV + t
        qT = sb.tile([128, 4 * NTOK], BF16, tag="qT")    # col co*NTOK + t
        kT = sb.tile([128, 4 * NKV], BF16, tag="kT")     # col co*NKV + t
        aT = sb.tile([128, 4 * NTOK], BF16, tag="aT")    # col ct*NTOK + q
        v_tok = [sb.tile([128, 520], BF16, tag=f"vtok{i}", name=f"vtok{i}")
                 for i in range(17)]                     # col h*65: [64 v | 1]
        maskB = sb.tile([128, NKV], BF16, tag="maskB")
        bandat = sb.tile([128, 128], BF16, tag="bandat")
        bandbt = sb.tile([128, 64], BF16, tag="bandbt")
        identj = sb.tile([128, 512], BF16, tag="identj")
        identj32 = sb.tile([128, 256], BF16, tag="identj32")
        ident = sb.tile([128, 128], BF16, tag="ident")
        wq = [sb.tile([128, C], BF16, tag=f"wq{i}", name=f"wq{i}") for i in range(4)]
        wk = [sb.tile([128, C], BF16, tag=f"wk{i}", name=f"wk{i}") for i in range(4)]
        wv = [sb.tile([128, C], BF16, tag=f"wv{i}", name=f"wv{i}") for i in range(4)]
        wp = [sb.tile([128, C], BF16, tag=f"wp{i}", name=f"wp{i}") for i in range(4)]
        bq_t = sb.tile([128, 4], F32, tag="bq")       # per-partition q bias
        bk_t = sb.tile([128, 4], F32, tag="bk")       # per-partition k bias
        bvB = sb.tile([128, C], F32, tag="bvB")       # v bias bcast over partitions
        bpB = sb.tile([128, C], F32, tag="bpB")       # proj bias bcast
        mq = sb.tile([128, NB], F32, tag="mq")        # query-token mask, per block

        # ---- constants / weights / x in (DMA only; no staging casts) ----
        nc.sync.dma_start(bandat[:], bandatd)
        nc.sync.dma_start(bandbt[:], bandbtd)
        nc.sync.dma_start(identj[:], identjd)
        nc.sync.dma_start(identj32[:], identj32d)
        nc.sync.dma_start(ident[:], identd)
        nc.sync.dma_start(bq_t[:], bqd.rearrange("(a b) -> b a", b=128))
        nc.sync.dma_start(bk_t[:], bkvd[0:C].rearrange("(a b) -> b a", b=128))
        nc.sync.dma_start(bvB[:], bkvd[C:2 * C][None, :].broadcast_to((128, C)))
        nc.sync.dma_start(bpB[:], bpd[None, :].broadcast_to((128, C)))
        nc.sync.dma_start(mq[:], maskd[PAD:PAD + NTOK].rearrange("(a b) -> b a", b=128))
        nc.sync.dma_start(maskB[:], maskbd[None, :].broadcast_to((128, NKV)))
        for ci in range(4):
            nc.sync.dma_start(wq[ci][:], wqd[ci * 128:(ci + 1) * 128, :])
            nc.sync.dma_start(wk[ci][:], wkd[ci * 128:(ci + 1) * 128, :])
            nc.sync.dma_start(wv[ci][:], wvd[ci * 128:(ci + 1) * 128, :])
            nc.sync.dma_start(wp[ci][:], wpd[ci * 128:(ci + 1) * 128, :])
            for hf in range(2):
                c0, c1 = hf * 1056, (hf + 1) * 1056
                nc.sync.dma_start(xT[:, ci * NKV + c0:ci * NKV + c1],
                                  xtd[ci * 128:(ci + 1) * 128, c0:c1])

        def emit_mask(g):
            # mask columns [528g, 528(g+1)) of all four c-chunks
            c0 = 528 * g
            xv = xT.rearrange("p (ci t) -> p ci t", ci=4)
            nc.vector.scalar_tensor_tensor(
                xv[:, :, c0:c0 + 528], xv[:, :, c0:c0 + 528], 1.0,
                maskB[:, c0:c0 + 528].unsqueeze(1).broadcast_to((128, 4, 528)),
                op0=mybir.AluOpType.mult, op1=mybir.AluOpType.mult)

        # Alternate projection-phase PSUM tiles between the two big pools so
        # consecutive chunks double-buffer (each pool alone has bufs=1).
        pcnt = [0]

        def proj_ps():
            pool = pp_sc if pcnt[0] % 2 == 0 else pp_av
            pcnt[0] += 1
            shape = [128, 2048] if pool is pp_sc else [128, 1024]
            return pool.tile(shape, F32, tag="sc" if pool is pp_sc else "av",
                             name=f"pps{pcnt[0]}")

        KCH = [512, 512, 512, 512, 64]

        def emit_kT_co(ch, co):
            t0 = 512 * ch
            w = KCH[ch]
            ps = proj_ps()
            for ci in range(4):
                nc.tensor.matmul(
                    ps[:, 0:w], wk[ci][:, co * 128:(co + 1) * 128],
                    xT[:, ci * NKV + t0:ci * NKV + t0 + w],
                    start=(ci == 0), stop=(ci == 3))
            nc.scalar.activation(kT[:, co * NKV + t0:co * NKV + t0 + w],
                                 ps[:, 0:w],
                                 mybir.ActivationFunctionType.Identity,
                                 bias=bk_t[:, co:co + 1])

        def emit_qT_co(ch, co):
            t0 = 512 * ch
            ps = proj_ps()
            for ci in range(4):
                nc.tensor.matmul(
                    ps[:, 0:512], wq[ci][:, co * 128:(co + 1) * 128],
                    xT[:, ci * NKV + PAD + t0:ci * NKV + PAD + t0 + 512],
                    start=(ci == 0), stop=(ci == 3))
            nc.scalar.activation(qT[:, co * NTOK + t0:co * NTOK + t0 + 512],
                                 ps[:, 0:512],
                                 mybir.ActivationFunctionType.Identity,
                                 bias=bq_t[:, co:co + 1])

        def emit_v(t):
            r0, r1 = t * 128, min((t + 1) * 128, NKV)
            rows = r1 - r0
            ps = proj_ps()
            for ci in range(4):
                nc.tensor.matmul(
                    ps[:rows, 0:512], xT[:, ci * NKV + r0:ci * NKV + r1],
                    wv[ci][:], start=(ci == 0), stop=(ci == 3))
            vv = v_tok[t].rearrange("p (h y) -> p h y", h=8)
            nc.gpsimd.memset(vv[:, :, 64:65], 1.0)
            nc.vector.scalar_tensor_tensor(
                vv[:rows, :, 0:64],
                ps[:rows, 0:512].rearrange("p (h y) -> p h y", h=8),
                1.0,
                bvB.rearrange("p (h y) -> p h y", h=8)[:rows],
                op0=mybir.AluOpType.mult, op1=mybir.AluOpType.add)



        # ---- attention: per 128-query block, software-pipelined ----
        # PE executes in program order, so emit block i-1's transpose/proj
        # between block i's scores and AV: by then their inputs (avn, aT)
        # are long ready, and they fill the PE while block i's exp runs.
        KPH = int(os.environ.get("KPH", "5"))

        avn_t = [None] * NB

        def stage_scores(i):
            sc = pp_sc.tile([128, 2048], F32, tag="sc", name=f"sc{i}")
            # Heads grouped by operand partition base per PSUM bank (a
            # matmul's tile_position row must be uniform within a bank):
            # even heads (base 0) fill bank 0/2, odd heads (base 64) 1/3.
            # Each bank is one accumulation group: head scores write
            # disjoint column ranges, then one additive band-mask matmul
            # (band @ [I I..]) over the whole bank; out-of-band scores
            # drop to ~-2000 so exp underflows to exactly 0.
            for b in range(2):
                for j in range(4):
                    h = 2 * j + b
                    co, hr = h // 2, (h % 2) * 64
                    nc.tensor.matmul(
                        sc[:, b * 512 + j * 128:b * 512 + (j + 1) * 128],
                        kT[hr:hr + 64, co * NKV + i * 128:co * NKV + i * 128 + 128],
                        qT[hr:hr + 64, co * NTOK + i * 128:co * NTOK + (i + 1) * 128],
                        start=(j == 0), stop=False, skip_group_check=True)
                nc.tensor.matmul(
                    sc[:, b * 512:(b + 1) * 512], bandat[:], identj[:],
                    start=False, stop=True, skip_group_check=True)
            for b in range(2):
                for j in range(4):
                    h = 2 * j + b
                    co, hr = h // 2, (h % 2) * 64
                    c0 = 1024 + b * 512 + j * 64
                    nc.tensor.matmul(
                        sc[0:64, c0:c0 + 64],
                        kT[hr:hr + 64, co * NKV + i * 128 + 128:co * NKV + i * 128 + 192],
                        qT[hr:hr + 64, co * NTOK + i * 128 + 64:co * NTOK + i * 128 + 128],
                        start=(j == 0), stop=False, skip_group_check=True)
                nc.tensor.matmul(
                    sc[0:64, 1024 + b * 512:1280 + b * 512], bandbt[:],
                    identj32[:], start=False, stop=True,
                    skip_group_check=True)
            alpha = sb_a.tile([128, 1536], BF16, tag="alpha", name=f"al{i}")
            for b in range(2):
                nc.scalar.activation(alpha[:, b * 512:(b + 1) * 512],
                                     sc[:, b * 512:(b + 1) * 512],
                                     mybir.ActivationFunctionType.Exp, scale=SCALE)
            for b in range(2):
                nc.scalar.activation(alpha[0:64, 1024 + b * 256:1280 + b * 256],
                                     sc[0:64, 1024 + b * 512:1280 + b * 512],
                                     mybir.ActivationFunctionType.Exp, scale=SCALE)
            return alpha

        def stage_av(i, alpha):
            av = pp_av.tile([128, 1024], F32, tag="av", name=f"av{i}")
            for h in range(8):
                c0 = (h // 4) * 512 + (h % 4) * 65
                ac = (h % 2) * 512 + (h // 2) * 128
                bc = 1024 + (h % 2) * 256 + (h // 2) * 64
                nc.tensor.matmul(
                    av[:, c0:c0 + 65],
                    alpha[:, ac:ac + 128],
                    v_tok[i][:, h * 65:h * 65 + 65],
                    start=True, stop=False, skip_group_check=True)
                nc.tensor.matmul(
                    av[64:128, c0:c0 + 65],
                    alpha[0:64, bc:bc + 64],
                    v_tok[i + 1][0:64, h * 65:h * 65 + 65],
                    start=False, stop=True, skip_group_check=True)
            avv = (av.rearrange("p (a c) -> p a c", a=2)[:, :, 0:260]
                     .rearrange("p a (h y) -> p a h y", h=4))
            rden = sb_o.tile([128, 8], F32, tag="rden", name=f"rd{i}")
            nc.vector.reciprocal(rden.rearrange("p (a h) -> p a h", a=2),
                                 avv[:, :, :, 64:65].squeeze(3))
            avn = sb_o.tile([128, 512], BF16, tag="avn", name=f"avn{i}")
            for a in range(2):
                nc.vector.scalar_tensor_tensor(
                    avn[:, a * 256:(a + 1) * 256]
                       .rearrange("p (h y) -> p h y", h=4),
                    avv[:, a:a + 1, :, 0:64].squeeze(1), mq[:, i:i + 1],
                    rden[:, a * 4:(a + 1) * 4].unsqueeze(2)
                        .broadcast_to((128, 4, 64)),
                    op0=mybir.AluOpType.mult, op1=mybir.AluOpType.mult)
            avn_t[i] = avn

        def stage_out(i):
            avn = avn_t[i]
            tr = pp_tr.tile([128, 512], BF16, tag="tr", name=f"tr{i}")
            for ct in range(4):
                nc.tensor.transpose(
                    tr[:, ct * 128:(ct + 1) * 128],
                    avn[:, ct * 128:(ct + 1) * 128],
                    ident[:])
            nc.scalar.activation(
                aT.rearrange("p (a c) -> p a c", a=4)[:, :, i * 128:(i + 1) * 128],
                tr.rearrange("p (a c) -> p a c", a=4),
                mybir.ActivationFunctionType.Copy)
            pr = pp_pr.tile([128, 512], F32, tag="pr", name=f"pr{i}")
            for ct in range(4):
                nc.tensor.matmul(
                    pr[:], aT[:, ct * NTOK + i * 128:ct * NTOK + (i + 1) * 128],
                    wp[ct][:], start=(ct == 0), stop=(ct == 3))
            ot = sb_o.tile([128, C], F32, tag="ot", name=f"ot{i}")
            nc.vector.scalar_tensor_tensor(
                ot[:], bpB[:], mq[:, i:i + 1], pr[:],
                op0=mybir.AluOpType.mult, op1=mybir.AluOpType.add)
            nc.sync.dma_start(outd[i * 128:(i + 1) * 128, :], ot[:])

        def emit_block(i):
            alpha = stage_scores(i)
            if KPH >= 4 and i >= 1:
                stage_out(i - 1)
            if KPH >= 3:
                stage_av(i, alpha)

        # Group 0 of the projections runs alone; groups 1..3 and the kT tail
        # interleave with attention blocks so dense N=512 GEMM streams pepper
        # the attention phase (keeps the HAM clock at 2.4GHz and fills
        # cross-engine stall windows).
        if int(os.environ.get("KPH", "5")) >= 1:
            emit_mask(0)
            for co in range(4):
                emit_kT_co(0, co)
            for co in range(4):
                emit_qT_co(0, co)
            for t in range(5):
                emit_v(t)
            for g in range(1, 4):
                units = ([lambda g=g: emit_mask(g)]
                         + [lambda g=g, co=co: emit_kT_co(g, co) for co in range(4)]
                         + [lambda g=g, co=co: emit_qT_co(g, co) for co in range(4)]
                         + [lambda t=t: emit_v(t) for t in range(4 * g + 1, 4 * g + 5)])
                blocks = list(range(4 * (g - 1), 4 * g)) if KPH >= 2 else []
                ui = 0
                for bi, i in enumerate(blocks):
                    take = 3 if bi < 3 else len(units) - ui
                    for u in units[ui:ui + take]:
                        u()
                    ui += take
                    emit_block(i)
                for u in units[ui:]:
                    u()
            tail_units = [lambda co=co: emit_kT_co(4, co) for co in range(4)]
            if KPH >= 2:
                for bi, i in enumerate(range(12, 16)):
                    if bi < len(tail_units):
                        tail_units[bi]()
                    emit_block(i)
            else:
                for u in tail_units:
                    u()
            if KPH >= 4:
                stage_out(NB - 1)

    nc.compile()
    return nc


_CACHE: dict = {}


def _get_program() -> bacc.Bacc:
    if "nc" not in _CACHE:
        _CACHE["nc"] = build_program()
    return _CACHE["nc"]


def _core_inputs(x, mask, Wq, bq, Wkv, bkv, Wproj, bproj):
    """Host-side marshalling: halo-slice, transpose, cast. Returns the
    per-core input maps."""
    consts = _mask_consts()
    wq8 = np.ascontiguousarray(np.asarray(Wq, np.float32).astype(ml_dtypes.bfloat16))
    wkv = np.asarray(Wkv, np.float32)
    wk8 = np.ascontiguousarray(wkv[:, 0:C].astype(ml_dtypes.bfloat16))
    wv8 = np.ascontiguousarray(wkv[:, C:2 * C].astype(ml_dtypes.bfloat16))
    wp8 = np.ascontiguousarray(np.asarray(Wproj, np.float32).astype(ml_dtypes.bfloat16))
    shared = {
        "wq": wq8, "wk": wk8, "wv": wv8, "wproj": wp8,
        "bq": np.asarray(bq, np.float32), "bkv": np.asarray(bkv, np.float32),
        "bproj": np.asarray(bproj, np.float32),
        "ident": np.ascontiguousarray(_identity()), **consts,
    }
    in_maps = []
    for core in range(8):
        b, h = divmod(core, 2)
        s = h * NTOK
        xc = np.zeros((NKV, C), np.float32)
        mc = np.zeros((NKV,), np.float32)
        lo, hi = max(0, s - PAD), min(T, s + NTOK + PAD)
        xc[lo - (s - PAD):lo - (s - PAD) + hi - lo] = x[b, lo:hi]
        mc[lo - (s - PAD):lo - (s - PAD) + hi - lo] = mask[b, lo:hi]
        in_maps.append({
            "xt": np.ascontiguousarray(xc.T.astype(ml_dtypes.bfloat16)),
            "mask": mc,
            "maskb": mc.astype(ml_dtypes.bfloat16),
            **shared,
        })
    return in_maps


def kernel(x, mask, Wq, bq, Wkv, bkv, Wproj, bproj) -> np.ndarray:
    x = np.asarray(x, np.float32)
    mask = np.asarray(mask, np.float32)
    nc = _get_program()
    in_maps = _core_inputs(x, mask, Wq, bq, Wkv, bkv, Wproj, bproj)
    res = bass_utils.run_bass_kernel_spmd(nc, in_maps, core_ids=list(range(8)))
    out = np.empty((B, T, C), np.float32)
    for core in range(8):
        b, h = divmod(core, 2)
        out[b, h * NTOK:(h + 1) * NTOK] = res.results[core]["out"]
    return out


# revision 36
# speedup vs baseline: 1.0499x; 1.0499x over previous
"""Local (sliding-window) self-attention Bass kernel for 8 TRN2 NeuronCores.

Problem: B=4, T=4096, C=512, H=8 heads, head_dim=64, window=15.
Sharding: 8 cores = batch(4) x seq-halves(2). Each core processes 2048 query
tokens of one batch element; its x chunk carries a 7-token halo on each side
(zero-padded at sequence edges, matching the reference's jnp.pad semantics),
padded to 2112 rows so the per-block k-tail chunk (64 wide) stays in range.

Host marshalling: x arrives pre-transposed (feature-major, bf16) and the
weights pre-cast to bf16 and pre-split, so the device only DMAs, applies the
mask, and runs matmuls.

Device dataflow (per core; all matmuls bf16 with fp32 PSUM):
  xT [128, 4*NKV] <- DMA; xT *= maskB (4 DVE ops, mask broadcast to 128 rows)
  qT/kT = W-stationary matmuls + bias (feature-major, via ACT copy)
  v_tok = xT-stationary matmuls + bias, token-major, packed [64 v | 1 ones]
          per head (the ones column makes AV emit the softmax denominator)
  per 128-query block i (queries qb=128i..qb+128):
    scoresT [k, q] per head: A chunk k in [qb, qb+128) (one matmul per head,
      even heads -> PSUM bank 0, odd -> bank 1: a matmul's tile_position row
      must be uniform within a bank), B tail k in [qb+128, qb+192) for
      queries qb+64.. (bank 2 even / bank 3 odd heads)
    additive band mask accumulated on the PE (band @ [I I I I]) drives
      out-of-band scores to -2000 so exp underflows to exactly 0
    exp per bank -> alpha bf16 (no DVE masking needed)
    AV token-major: av[q, 65] per head = alpha.T @ v_aug (den in col 64)
    reciprocal(den) * query-mask -> one DVE normalize -> avn bf16
    4 PE transposes -> attnT -> one ACT copy -> aT
    proj: aT-stationary matmuls; out = (bproj*mask) + psum via one DVE op
"""

import math
import os
from contextlib import ExitStack

import ml_dtypes
import numpy as np

import concourse.bacc as bacc
import concourse.bass as bass
import concourse.mybir as mybir
import concourse.tile as tile
from concourse import bass_utils

B, T, C, H, WIN = 4, 4096, 512, 8, 15
D = C // H            # 64
PAD = WIN // 2        # 7
NTOK = T // 2         # 2048 query tokens per core
NKV = 2112            # kv rows per core: 7 + 2048 + 7 = 2062, padded to 2112
NB = NTOK // 128      # 16 query blocks
SCALE = math.log(WIN) / D
F32 = mybir.dt.float32
BF16 = mybir.dt.bfloat16
M0 = 2000.0   # additive score mask; SCALE*M0 ~ 85 so exp underflows to 0


def _mask_consts() -> dict:
    """Additive band masks (transposed, as matmul lhsT) and replicated
    identities used to broadcast them across the head-columns of a bank.

    A-chunk in-band: k-14 <= q <= k.  B-tail (k=128+k'): q64 >= 50+k'
    where q = qb+64+q64.  B operands are zero-padded to K=128 so the mask
    matmul shares the full-array row config of the A-mask.
    """
    k = np.arange(128)[:, None]
    q = np.arange(128)[None, :]
    a = np.where((q >= k - 14) & (q <= k), 0.0, -M0).astype(np.float32)
    kp = np.arange(64)[:, None]
    q64 = np.arange(64)[None, :]
    b = np.where(q64 >= 50 + kp, 0.0, -M0).astype(np.float32)
    identj = np.zeros((128, 512), np.float32)
    for j in range(4):
        identj[:, j * 128:(j + 1) * 128] = np.eye(128)
    identj32 = np.zeros((128, 256), np.float32)
    for j in range(4):
        identj32[0:64, j * 64:(j + 1) * 64] = np.eye(64)
    return {
        "bandat": np.ascontiguousarray(a.T.astype(ml_dtypes.bfloat16)),
        "bandbt": np.ascontiguousarray(
            np.vstack([b.T, np.zeros((64, 64), np.float32)])
              .astype(ml_dtypes.bfloat16)),
        "identj": np.ascontiguousarray(identj.astype(ml_dtypes.bfloat16)),
        "identj32": np.ascontiguousarray(identj32.astype(ml_dtypes.bfloat16)),
    }


def _identity() -> np.ndarray:
    return np.eye(128, dtype=ml_dtypes.bfloat16)


def build_program() -> bacc.Bacc:
    nc = bacc.Bacc("TRN2", target_bir_lowering=False, debug=False,
                   enable_asserts=False, num_devices=8)

    xtd = nc.dram_tensor("xt", [C, NKV], BF16, kind="ExternalInput").ap()
    maskd = nc.dram_tensor("mask", [NKV], F32, kind="ExternalInput").ap()
    maskbd = nc.dram_tensor("maskb", [NKV], BF16, kind="ExternalInput").ap()
    wqd = nc.dram_tensor("wq", [C, C], BF16, kind="ExternalInput").ap()
    bqd = nc.dram_tensor("bq", [C], F32, kind="ExternalInput").ap()
    wkd = nc.dram_tensor("wk", [C, C], BF16, kind="ExternalInput").ap()
    wvd = nc.dram_tensor("wv", [C, C], BF16, kind="ExternalInput").ap()
    bkvd = nc.dram_tensor("bkv", [2 * C], F32, kind="ExternalInput").ap()
    wpd = nc.dram_tensor("wproj", [C, C], BF16, kind="ExternalInput").ap()
    bpd = nc.dram_tensor("bproj", [C], F32, kind="ExternalInput").ap()
    bandatd = nc.dram_tensor("bandat", [128, 128], BF16, kind="ExternalInput").ap()
    bandbtd = nc.dram_tensor("bandbt", [128, 64], BF16, kind="ExternalInput").ap()
    identjd = nc.dram_tensor("identj", [128, 512], BF16, kind="ExternalInput").ap()
    identj32d = nc.dram_tensor("identj32", [128, 256], BF16, kind="ExternalInput").ap()
    identd = nc.dram_tensor("ident", [128, 128], BF16, kind="ExternalInput").ap()
    outd = nc.dram_tensor("out", [NTOK, C], F32, kind="ExternalOutput").ap()

    with tile.TileContext(nc) as tc, ExitStack() as ctx:
        sb = ctx.enter_context(tc.tile_pool(name="sb", bufs=1))
        sb_a = ctx.enter_context(tc.tile_pool(name="sb_a", bufs=3))
        sb_o = ctx.enter_context(tc.tile_pool(name="sb_o", bufs=3))
        pp_sc = ctx.enter_context(tc.tile_pool(name="pp_sc", bufs=1, space="PSUM"))
        pp_tr = ctx.enter_context(tc.tile_pool(name="pp_tr", bufs=1, space="PSUM"))
        pp_pr = ctx.enter_context(tc.tile_pool(name="pp_pr", bufs=1, space="PSUM"))
        pp_av = ctx.enter_context(tc.tile_pool(name="pp_av", bufs=1, space="PSUM"))

        # ---- persistent SBUF tensors ----
        xT = sb.tile([128, 4 * NKV], BF16, tag="xT")     # col ci*NKV + t
        qT = sb.tile([128, 4 * NTOK], BF16, tag="qT")    # col co*NTOK + t
        kT = sb.tile([128, 4 * NKV], BF16, tag="kT")     # col co*NKV + t
        aT = sb.tile([128, 4 * NTOK], BF16, tag="aT")    # col ct*NTOK + q
        v_tok = [sb.tile([128, 520], BF16, tag=f"vtok{i}", name=f"vtok{i}")
                 for i in range(17)]                     # col h*65: [64 v | 1]
        maskB = sb.tile([128, NKV], BF16, tag="maskB")
        bandat = sb.tile([128, 128], BF16, tag="bandat")
        bandbt = sb.tile([128, 64], BF16, tag="bandbt")
        identj = sb.tile([128, 512], BF16, tag="identj")
        identj32 = sb.tile([128, 256], BF16, tag="identj32")
        ident = sb.tile([128, 128], BF16, tag="ident")
        wq = [sb.tile([128, C], BF16, tag=f"wq{i}", name=f"wq{i}") for i in range(4)]
        wk = [sb.tile([128, C], BF16, tag=f"wk{i}", name=f"wk{i}") for i in range(4)]
        wv = [sb.tile([128, C], BF16, tag=f"wv{i}", name=f"wv{i}") for i in range(4)]
        wp = [sb.tile([128, C], BF16, tag=f"wp{i}", name=f"wp{i}") for i in range(4)]
        bq_t = sb.tile([128, 4], F32, tag="bq")       # per-partition q bias
        bk_t = sb.tile([128, 4], F32, tag="bk")       # per-partition k bias
        bvB = sb.tile([128, C], F32, tag="bvB")       # v bias bcast over partitions
        bpB = sb.tile([128, C], F32, tag="bpB")       # proj bias bcast
        mq = sb.tile([128, NB], F32, tag="mq")        # query-token mask, per block

        # ---- constants / weights / x in (DMA only; no staging casts) ----
        nc.sync.dma_start(bandat[:], bandatd)
        nc.sync.dma_start(bandbt[:], bandbtd)
        nc.sync.dma_start(identj[:], identjd)
        nc.sync.dma_start(identj32[:], identj32d)
        nc.sync.dma_start(ident[:], identd)
        nc.sync.dma_start(bq_t[:], bqd.rearrange("(a b) -> b a", b=128))
        nc.sync.dma_start(bk_t[:], bkvd[0:C].rearrange("(a b) -> b a", b=128))
        nc.sync.dma_start(bvB[:], bkvd[C:2 * C][None, :].broadcast_to((128, C)))
        nc.sync.dma_start(bpB[:], bpd[None, :].broadcast_to((128, C)))
        nc.sync.dma_start(mq[:], maskd[PAD:PAD + NTOK].rearrange("(a b) -> b a", b=128))
        nc.sync.dma_start(maskB[:], maskbd[None, :].broadcast_to((128, NKV)))
        for ci in range(4):
            nc.sync.dma_start(wq[ci][:], wqd[ci * 128:(ci + 1) * 128, :])
            nc.sync.dma_start(wk[ci][:], wkd[ci * 128:(ci + 1) * 128, :])
            nc.sync.dma_start(wv[ci][:], wvd[ci * 128:(ci + 1) * 128, :])
            nc.sync.dma_start(wp[ci][:], wpd[ci * 128:(ci + 1) * 128, :])
            for hf in range(2):
                c0, c1 = hf * 1056, (hf + 1) * 1056
                nc.sync.dma_start(xT[:, ci * NKV + c0:ci * NKV + c1],
                                  xtd[ci * 128:(ci + 1) * 128, c0:c1])

        def emit_mask(g):
            # mask columns [528g, 528(g+1)) of all four c-chunks
            c0 = 528 * g
            xv = xT.rearrange("p (ci t) -> p ci t", ci=4)
            nc.vector.scalar_tensor_tensor(
                xv[:, :, c0:c0 + 528], xv[:, :, c0:c0 + 528], 1.0,
                maskB[:, c0:c0 + 528].unsqueeze(1).broadcast_to((128, 4, 528)),
                op0=mybir.AluOpType.mult, op1=mybir.AluOpType.mult)

        # Alternate projection-phase PSUM tiles between the two big pools so
        # consecutive chunks double-buffer (each pool alone has bufs=1).
        pcnt = [0]

        def proj_ps():
            pool = pp_sc if pcnt[0] % 2 == 0 else pp_av
            pcnt[0] += 1
            shape = [128, 2048] if pool is pp_sc else [128, 1024]
            return pool.tile(shape, F32, tag="sc" if pool is pp_sc else "av",
                             name=f"pps{pcnt[0]}")

        KCH = [512, 512, 512, 512, 64]

        def emit_kT_co(ch, co):
            t0 = 512 * ch
            w = KCH[ch]
            ps = proj_ps()
            for ci in range(4):
                nc.tensor.matmul(
                    ps[:, 0:w], wk[ci][:, co * 128:(co + 1) * 128],
                    xT[:, ci * NKV + t0:ci * NKV + t0 + w],
                    start=(ci == 0), stop=(ci == 3))
            nc.scalar.activation(kT[:, co * NKV + t0:co * NKV + t0 + w],
                                 ps[:, 0:w],
                                 mybir.ActivationFunctionType.Identity,
                                 bias=bk_t[:, co:co + 1])

        def emit_qT_co(ch, co):
            t0 = 512 * ch
            ps = proj_ps()
            for ci in range(4):
                nc.tensor.matmul(
                    ps[:, 0:512], wq[ci][:, co * 128:(co + 1) * 128],
                    xT[:, ci * NKV + PAD + t0:ci * NKV + PAD + t0 + 512],
                    start=(ci == 0), stop=(ci == 3))
            nc.scalar.activation(qT[:, co * NTOK + t0:co * NTOK + t0 + 512],
                                 ps[:, 0:512],
                                 mybir.ActivationFunctionType.Identity,
                                 bias=bq_t[:, co:co + 1])

        def emit_v(t):
            r0, r1 = t * 128, min((t + 1) * 128, NKV)
            rows = r1 - r0
            ps = proj_ps()
            for ci in range(4):
                nc.tensor.matmul(
                    ps[:rows, 0:512], xT[:, ci * NKV + r0:ci * NKV + r1],
                    wv[ci][:], start=(ci == 0), stop=(ci == 3))
            vv = v_tok[t].rearrange("p (h y) -> p h y", h=8)
            nc.gpsimd.memset(vv[:, :, 64:65], 1.0)
            nc.vector.scalar_tensor_tensor(
                vv[:rows, :, 0:64],
                ps[:rows, 0:512].rearrange("p (h y) -> p h y", h=8),
                1.0,
                bvB.rearrange("p (h y) -> p h y", h=8)[:rows],
                op0=mybir.AluOpType.mult, op1=mybir.AluOpType.add)



        # ---- attention: per 128-query block, software-pipelined ----
        # PE executes in program order, so emit block i-1's transpose/proj
        # between block i's scores and AV: by then their inputs (avn, aT)
        # are long ready, and they fill the PE while block i's exp runs.
        KPH = int(os.environ.get("KPH", "5"))

        avn_t = [None] * NB

        def stage_scores(i):
            sc = pp_sc.tile([128, 2048], F32, tag="sc", name=f"sc{i}")
            # Heads grouped by operand partition base per PSUM bank (a
            # matmul's tile_position row must be uniform within a bank):
            # even heads (base 0) fill bank 0/2, odd heads (base 64) 1/3.
            # Each bank is one accumulation group: head scores write
            # disjoint column ranges, then one additive band-mask matmul
            # (band @ [I I..]) over the whole bank; out-of-band scores
            # drop to ~-2000 so exp underflows to exactly 0.
            for b in range(2):
                for j in range(4):
                    h = 2 * j + b
                    co, hr = h // 2, (h % 2) * 64
                    nc.tensor.matmul(
                        sc[:, b * 512 + j * 128:b * 512 + (j + 1) * 128],
                        kT[hr:hr + 64, co * NKV + i * 128:co * NKV + i * 128 + 128],
                        qT[hr:hr + 64, co * NTOK + i * 128:co * NTOK + (i + 1) * 128],
                        start=(j == 0), stop=False, skip_group_check=True)
                nc.tensor.matmul(
                    sc[:, b * 512:(b + 1) * 512], bandat[:], identj[:],
                    start=False, stop=True, skip_group_check=True)
            for b in range(2):
                for j in range(4):
                    h = 2 * j + b
                    co, hr = h // 2, (h % 2) * 64
                    c0 = 1024 + b * 512 + j * 64
                    nc.tensor.matmul(
                        sc[0:64, c0:c0 + 64],
                        kT[hr:hr + 64, co * NKV + i * 128 + 128:co * NKV + i * 128 + 192],
                        qT[hr:hr + 64, co * NTOK + i * 128 + 64:co * NTOK + i * 128 + 128],
                        start=(j == 0), stop=False, skip_group_check=True)
                nc.tensor.matmul(
                    sc[0:64, 1024 + b * 512:1280 + b * 512], bandbt[:],
                    identj32[:], start=False, stop=True,
                    skip_group_check=True)
            alpha = sb_a.tile([128, 1536], BF16, tag="alpha", name=f"al{i}")
            for b in range(2):
                nc.scalar.activation(alpha[:, b * 512:(b + 1) * 512],
                                     sc[:, b * 512:(b + 1) * 512],
                                     mybir.ActivationFunctionType.Exp, scale=SCALE)
            for b in range(2):
                nc.scalar.activation(alpha[0:64, 1024 + b * 256:1280 + b * 256],
                                     sc[0:64, 1024 + b * 512:1280 + b * 512],
                                     mybir.ActivationFunctionType.Exp, scale=SCALE)
            return alpha

        def stage_av(i, alpha):
            av = pp_av.tile([128, 1024], F32, tag="av", name=f"av{i}")
            for h in range(8):
                c0 = (h // 4) * 512 + (h % 4) * 65
                ac = (h % 2) * 512 + (h // 2) * 128
                bc = 1024 + (h % 2) * 256 + (h // 2) * 64
                nc.tensor.matmul(
                    av[:, c0:c0 + 65],
                    alpha[:, ac:ac + 128],
                    v_tok[i][:, h * 65:h * 65 + 65],
                    start=True, stop=False, skip_group_check=True)
                nc.tensor.matmul(
                    av[64:128, c0:c0 + 65],
                    alpha[0:64, bc:bc + 64],
                    v_tok[i + 1][0:64, h * 65:h * 65 + 65],
                    start=False, stop=True, skip_group_check=True)
            avv = (av.rearrange("p (a c) -> p a c", a=2)[:, :, 0:260]
                     .rearrange("p a (h y) -> p a h y", h=4))
            rden = sb_o.tile([128, 8], F32, tag="rden", name=f"rd{i}")
            nc.vector.reciprocal(rden.rearrange("p (a h) -> p a h", a=2),
                                 avv[:, :, :, 64:65].squeeze(3))
            avn = sb_o.tile([128, 512], BF16, tag="avn", name=f"avn{i}")
            for a in range(2):
                nc.vector.scalar_tensor_tensor(
                    avn[:, a * 256:(a + 1) * 256]
                       .rearrange("p (h y) -> p h y", h=4),
                    avv[:, a:a + 1, :, 0:64].squeeze(1), mq[:, i:i + 1],
                    rden[:, a * 4:(a + 1) * 4].unsqueeze(2)
                        .broadcast_to((128, 4, 64)),
                    op0=mybir.AluOpType.mult, op1=mybir.AluOpType.mult)
            avn_t[i] = avn

        def stage_out(i):
            avn = avn_t[i]
            tr = pp_tr.tile([128, 512], BF16, tag="tr", name=f"tr{i}")
            for ct in range(4):
                nc.tensor.transpose(
                    tr[:, ct * 128:(ct + 1) * 128],
                    avn[:, ct * 128:(ct + 1) * 128],
                    ident[:])
            nc.scalar.activation(
                aT.rearrange("p (a c) -> p a c", a=4)[:, :, i * 128:(i + 1) * 128],
                tr.rearrange("p (a c) -> p a c", a=4),
                mybir.ActivationFunctionType.Copy)
            pr = pp_pr.tile([128, 512], F32, tag="pr", name=f"pr{i}")
            for ct in range(4):
                nc.tensor.matmul(
                    pr[:], aT[:, ct * NTOK + i * 128:ct * NTOK + (i + 1) * 128],
                    wp[ct][:], start=(ct == 0), stop=(ct == 3))
            ot = sb_o.tile([128, C], F32, tag="ot", name=f"ot{i}")
            nc.vector.scalar_tensor_tensor(
                ot[:], bpB[:], mq[:, i:i + 1], pr[:],
                op0=mybir.AluOpType.mult, op1=mybir.AluOpType.add)
            nc.sync.dma_start(outd[i * 128:(i + 1) * 128, :], ot[:])

        if int(os.environ.get("KPH", "5")) >= 1:
            for g in range(4):
                emit_mask(g)
                for co in range(4):
                    emit_kT_co(g, co)
                for co in range(4):
                    emit_qT_co(g, co)
                for t in range(4 * g, 4 * g + 4):
                    emit_v(t)
            for co in range(4):
                emit_kT_co(4, co)
            emit_v(16)

        if KPH >= 2:
            for i in range(NB):
                alpha = stage_scores(i)
                if KPH >= 4 and i >= 1:
                    stage_out(i - 1)
                if KPH >= 3:
                    stage_av(i, alpha)
            if KPH >= 4:
                stage_out(NB - 1)

    nc.compile()
    return nc


_CACHE: dict = {}


def _get_program() -> bacc.Bacc:
    if "nc" not in _CACHE:
        _CACHE["nc"] = build_program()
    return _CACHE["nc"]


def _core_inputs(x, mask, Wq, bq, Wkv, bkv, Wproj, bproj):
    """Host-side marshalling: halo-slice, transpose, cast. Returns the
    per-core input maps."""
    consts = _mask_consts()
    wq8 = np.ascontiguousarray(np.asarray(Wq, np.float32).astype(ml_dtypes.bfloat16))
    wkv = np.asarray(Wkv, np.float32)
    wk8 = np.ascontiguousarray(wkv[:, 0:C].astype(ml_dtypes.bfloat16))
    wv8 = np.ascontiguousarray(wkv[:, C:2 * C].astype(ml_dtypes.bfloat16))
    wp8 = np.ascontiguousarray(np.asarray(Wproj, np.float32).astype(ml_dtypes.bfloat16))
    shared = {
        "wq": wq8, "wk": wk8, "wv": wv8, "wproj": wp8,
        "bq": np.asarray(bq, np.float32), "bkv": np.asarray(bkv, np.float32),
        "bproj": np.asarray(bproj, np.float32),
        "ident": np.ascontiguousarray(_identity()), **consts,
    }
    in_maps = []
    for core in range(8):
        b, h = divmod(core, 2)
        s = h * NTOK
        xc = np.zeros((NKV, C), np.float32)
        mc = np.zeros((NKV,), np.float32)
        lo, hi = max(0, s - PAD), min(T, s + NTOK + PAD)
        xc[lo - (s - PAD):lo - (s - PAD) + hi - lo] = x[b, lo:hi]
        mc[lo - (s - PAD):lo - (s - PAD) + hi - lo] = mask[b, lo:hi]
        in_maps.append({
            "xt": np.ascontiguousarray(xc.T.astype(ml_dtypes.bfloat16)),
            "mask": mc,
            "maskb": mc.astype(ml_dtypes.bfloat16),
            **shared,
        })
    return in_maps


def kernel(x, mask, Wq, bq, Wkv, bkv, Wproj, bproj) -> np.ndarray:
    x = np.asarray(x, np.float32)
    mask = np.asarray(mask, np.float32)
    nc = _get_program()
    in_maps = _core_inputs(x, mask, Wq, bq, Wkv, bkv, Wproj, bproj)
    res = bass_utils.run_bass_kernel_spmd(nc, in_maps, core_ids=list(range(8)))
    out = np.empty((B, T, C), np.float32)
    for core in range(8):
        b, h = divmod(core, 2)
        out[b, h * NTOK:(h + 1) * NTOK] = res.results[core]["out"]
    return out
